# revision 1
# baseline (speedup 1.0000x reference)
"""Trainium2 Bass kernel for nn_Attention_52604759441672.

Dense causal self-attention block (LayerNorm -> QKV -> RoPE -> causal
softmax attention -> output projection) for x of shape (2, 2048, 1024),
16 heads x 64 dim. Sharded over 8 NeuronCores: data parallel over the
2 batches x tensor parallel over 4 head-groups (4 heads each). Each core
computes its batch's LayerNorm, its head-group's QKV projections,
attention, and a partial output projection; the host sums the 4 partial
outputs per batch.

The whole kernel is a single software-pipelined pass over four 512-token
chunks: LN+transpose -> QKV -> RoPE -> attention -> output projection
stream chunk-by-chunk so the tensor engine never waits on a full phase.

Internal layouts (per core, fp32r = TF32-like full-rate PE dtype):
  xn^T   [128, 8, 512]    per-chunk d-major normalized activations
  ropeq  [128, 2, 512]    per-chunk, 2 head-pairs stacked (64 dims each)
  ropek  [128, 2, 2048]   persistent (all k tokens needed later)
  v_aug  [128, 16, 4, 65] token-major V per k-tile/head + ones column
                          (ones row gives the softmax denominator for free)
  scores are computed transposed: S^T[k, q] so the softmax sum runs through
  the PE via the ones column; exp runs on the scalar engine straight out
  of PSUM.
"""

import os
import sys

for _p in ("/opt/trn_rl_repo",):
    if _p not in sys.path and os.path.isdir(_p):
        sys.path.insert(0, _p)

import numpy as np

import concourse.bass as bass
import concourse.mybir as mybir
import concourse.tile as tile
from concourse import bacc, bass_utils

F32 = mybir.dt.float32
F32R = mybir.dt.float32r
AF = mybir.ActivationFunctionType
ALU = mybir.AluOpType

N_CORES = 8
N = 2048          # sequence length
DIM = 1024        # model dim
DH = 64           # head dim
HPC = 4           # heads per core
HG = HPC * DH     # head-group width = 256
NT = N // 128     # 16 token tiles
KC = DIM // 128   # 8 contraction chunks
CH = N // 512     # 4 q-chunks of 512
SCALE = DH ** -0.5

# engine choice for elementwise offloads (gpsimd frees the vector engine)
GP_MASK = True
GP_ROPE_ADD = True
GP_NORM = True

_CACHE = {}


def _rope_tables():
    inv_freq = 1.0 / (10000.0 ** (np.arange(0, DH, 2, dtype=np.float64) / DH))
    freqs = np.arange(N, dtype=np.float64)[:, None] * inv_freq[None, :]  # (N, 32)
    cos32 = np.cos(freqs).astype(np.float32).T     # (32, N)
    sin32 = np.sin(freqs).astype(np.float32).T     # (32, N)
    cos64 = np.concatenate([cos32, cos32], axis=0)             # (64, N)
    sin64sh = np.concatenate([sin32, -sin32], axis=0)          # pre-shuffled
    cos128 = np.ascontiguousarray(np.tile(cos64, (2, 1)))      # (128, N)
    sinsh128 = np.ascontiguousarray(np.tile(sin64sh, (2, 1)))
    return cos128, sinsh128


def build_nc():
    nc = bacc.Bacc("TRN2", target_bir_lowering=False, debug=False,
                   enable_asserts=True, num_devices=N_CORES)
    dt = nc.dram_tensor
    d = {
        "x": dt("x", [N, DIM], F32, kind="ExternalInput").ap(),
        "wq": dt("wq", [DIM, HG], F32, kind="ExternalInput").ap(),
        "wk": dt("wk", [DIM, HG], F32, kind="ExternalInput").ap(),
        "wv": dt("wv", [DIM, HG], F32, kind="ExternalInput").ap(),
        "wo": dt("wo", [HG, DIM], F32, kind="ExternalInput").ap(),
        "cos": dt("cos", [128, N], F32, kind="ExternalInput").ap(),
        "sinsh": dt("sinsh", [128, N], F32, kind="ExternalInput").ap(),
        "tri": dt("tri", [128, 128], F32, kind="ExternalInput").ap(),
        "ident": dt("ident", [128, 128], F32, kind="ExternalInput").ap(),
        "onez": dt("onez", [128, 384], F32, kind="ExternalInput").ap(),
        "out": dt("out", [N, DIM], F32, kind="ExternalOutput").ap(),
    }
    with tile.TileContext(nc) as tc:
        _emit(nc, tc, d)
    nc.compile()
    return nc


def _emit(nc, tc, d):
    from contextlib import ExitStack
    ctx = ExitStack()
    with ctx:
        consts = ctx.enter_context(tc.tile_pool(name="consts", bufs=1))
        wpool = ctx.enter_context(tc.tile_pool(name="wpool", bufs=1))
        persist = ctx.enter_context(tc.tile_pool(name="persist", bufs=1))
        tbl = ctx.enter_context(tc.tile_pool(name="tbl", bufs=2))
        xnp = ctx.enter_context(tc.tile_pool(name="xnp", bufs=2))
        rqp = ctx.enter_context(tc.tile_pool(name="rqp", bufs=2))
        cxp = ctx.enter_context(tc.tile_pool(name="cxp", bufs=2))
        ph1 = ctx.enter_context(tc.tile_pool(name="ph1", bufs=3))
        ph1s = ctx.enter_context(tc.tile_pool(name="ph1s", bufs=4))
        ph2 = ctx.enter_context(tc.tile_pool(name="ph2", bufs=2))
        ph3 = ctx.enter_context(tc.tile_pool(name="ph3", bufs=3))
        ph3s = ctx.enter_context(tc.tile_pool(name="ph3s", bufs=2))
        ph4 = ctx.enter_context(tc.tile_pool(name="ph4", bufs=3))
        dsc = ctx.enter_context(tc.tile_pool(name="dsc", bufs=8, space="DRAM"))
        # PSUM: scratch(2) + qkv(1) + s(3) + ctxA(1) + ctxB(1) = 8 banks
        scr_ps = ctx.enter_context(
            tc.tile_pool(name="scr_ps", bufs=2, space="PSUM"))
        qkv_ps = ctx.enter_context(
            tc.tile_pool(name="qkv_ps", bufs=1, space="PSUM"))
        s_ps = ctx.enter_context(tc.tile_pool(name="s_ps", bufs=3, space="PSUM"))
        ctxA_ps = ctx.enter_context(
            tc.tile_pool(name="ctxA_ps", bufs=1, space="PSUM"))
        ctxB_ps = ctx.enter_context(
            tc.tile_pool(name="ctxB_ps", bufs=1, space="PSUM"))

        # ---- small constants + first x tiles ahead of the big weights ----
        tri_sb = consts.tile([128, 128], F32)
        nc.sync.dma_start(out=tri_sb, in_=d["tri"])
        ident_sb = consts.tile([128, 128], F32R)
        nc.sync.dma_start(out=ident_sb, in_=d["ident"].bitcast(F32R))
        eps_sb = consts.tile([128, 1], F32)
        nc.vector.memset(eps_sb, 1e-5)
        zer_sb = consts.tile([128, 384], F32R)
        nc.vector.memset(zer_sb.bitcast(F32), 0.0)

        x_tiles = {}
        for it in range(3):
            x_t = ph1.tile([128, DIM], F32, name=f"x_t{it}", tag="x_t")
            nc.sync.dma_start(out=x_t, in_=d["x"][it * 128:(it + 1) * 128, :])
            x_tiles[it] = x_t

        # ---- big constants ----
        wq_sb = wpool.tile([128, KC, HG], F32R)
        nc.sync.dma_start(out=wq_sb, in_=d["wq"].bitcast(F32R).rearrange(
            "(kc p) f -> p kc f", p=128))
        wk_sb = wpool.tile([128, KC, HG], F32R)
        nc.sync.dma_start(out=wk_sb, in_=d["wk"].bitcast(F32R).rearrange(
            "(kc p) f -> p kc f", p=128))
        wv_sb = wpool.tile([128, KC, HG], F32R)
        nc.sync.dma_start(out=wv_sb, in_=d["wv"].bitcast(F32R).rearrange(
            "(kc p) f -> p kc f", p=128))
        wo_sb = wpool.tile([128, 2, DIM], F32R)
        nc.sync.dma_start(out=wo_sb, in_=d["wo"].bitcast(F32R).rearrange(
            "(c p) f -> p c f", p=128))

        ropek = persist.tile([128, 2, N], F32R)
        vaug = persist.tile([128, NT, HPC, DH + 1], F32R)
        # ones column of v_aug (memset cannot write fp32r; DMA from DRAM)
        nc.sync.dma_start(
            out=vaug[:, :, :, DH:DH + 1],
            in_=d["onez"].bitcast(F32R)[:, 0:NT * HPC].rearrange(
                "p (j h o) -> p j h o", j=NT, h=HPC))

        def _emit_wo(c, cx):
            # output projection for token tiles of chunk c (deferred one
            # pair-loop so the denominator DMA round-trip is off the PE path)
            for b4 in range(4):
                it = c * 4 + b4
                for nh in range(2):
                    op = scr_ps.tile([128, 512], F32, name="op", tag="scr")
                    for pc in range(2):
                        nc.tensor.matmul(
                            op, cx[:, pc, b4 * 128:(b4 + 1) * 128],
                            wo_sb[:, pc, nh * 512:(nh + 1) * 512],
                            start=(pc == 0), stop=(pc == 1))
                    ocp = ph4.tile([128, 512], F32, name="ocp", tag="ocp")
                    if nh == 0:
                        nc.vector.tensor_copy(ocp, op)
                    else:
                        nc.scalar.copy(ocp, op)
                    nc.sync.dma_start(
                        out=d["out"][it * 128:(it + 1) * 128,
                                     nh * 512:(nh + 1) * 512],
                        in_=ocp)

        pending_wo = None
        gp_mask = nc.gpsimd if GP_MASK else nc.vector
        gp_rope = nc.gpsimd if GP_ROPE_ADD else nc.vector
        gp_norm = nc.gpsimd if GP_NORM else nc.vector

        xncs = {}

        def _emit_ln_tp(c):
            # LayerNorm + PE transpose for token tiles of chunk c -> xn^T
            xnc = xnp.tile([128, KC, 512], F32R, name="xnc", tag="xnc")
            xncs[c] = xnc
            for b4 in range(4):
                it = c * 4 + b4
                if it in x_tiles:
                    x_t = x_tiles.pop(it)
                else:
                    x_t = ph1.tile([128, DIM], F32, name=f"x_t{it}", tag="x_t")
                    nc.sync.dma_start(out=x_t,
                                      in_=d["x"][it * 128:(it + 1) * 128, :])
                stats = ph1s.tile([128, 2, 6], F32, name="stats", tag="lns")
                nc.vector.bn_stats(out=stats[:, 0, :], in_=x_t[:, 0:512])
                nc.vector.bn_stats(out=stats[:, 1, :], in_=x_t[:, 512:1024])
                mv = ph1s.tile([128, 2], F32, name="mv", tag="lns")
                nc.vector.bn_aggr(out=mv, in_=stats)
                rstd = ph1s.tile([128, 1], F32, name="rstd", tag="lns")
                nc.scalar.activation(out=rstd, in_=mv[:, 1:2], func=AF.Sqrt,
                                     bias=eps_sb)
                nc.vector.reciprocal(out=rstd, in_=rstd)
                xn_t = ph1.tile([128, DIM], F32R, name="xn_t", tag="xn_t",
                                bufs=2)
                nc.vector.tensor_scalar(out=xn_t, in0=x_t,
                                        scalar1=mv[:, 0:1], scalar2=rstd,
                                        op0=ALU.subtract, op1=ALU.mult)
                for half in range(2):
                    tp = scr_ps.tile([128, 512], F32R, name="tp", tag="scr")
                    for b in range(4):
                        kc = half * 4 + b
                        nc.tensor.transpose(tp[:, b * 128:(b + 1) * 128],
                                            xn_t[:, kc * 128:(kc + 1) * 128],
                                            ident_sb)
                    dst = xnc[:, half * 4:(half + 1) * 4,
                              b4 * 128:(b4 + 1) * 128]
                    src = tp.rearrange("p (b f) -> p b f", b=4)
                    nc.scalar.copy(dst, src)

        _emit_ln_tp(0)
        for c in range(CH):
            cs = slice(c * 512, (c + 1) * 512)
            cos_c = tbl.tile([128, 512], F32, tag="cos_c")
            nc.sync.dma_start(out=cos_c, in_=d["cos"][:, cs])
            sinsh_c = tbl.tile([128, 512], F32, tag="sinsh_c")
            nc.sync.dma_start(out=sinsh_c, in_=d["sinsh"][:, cs])
            xnc = xncs.pop(c)
            rq = rqp.tile([128, 2, 512], F32R, tag="rq")
            cx = cxp.tile([128, 2, 512], F32R, tag="cx")

            # ---------- QKV chunk c + RoPE + V assembly ----------
            for kind, w_sb, of in (("v", wv_sb, 0), ("v", wv_sb, 1),
                                   ("q", wq_sb, 0), ("q", wq_sb, 1),
                                   ("k", wk_sb, 0), ("k", wk_sb, 1)):
                ps = qkv_ps.tile([128, 512], F32, name=f"qkvps_{kind}{of}",
                                 tag="qkvps")
                for kc in range(KC):
                    nc.tensor.matmul(
                        ps, w_sb[:, kc, of * 128:(of + 1) * 128],
                        xnc[:, kc, :], start=(kc == 0), stop=(kc == KC - 1))
                if kind in ("q", "k"):
                    ta = ph2.tile([128, 512], F32, tag="ta")
                    nc.vector.tensor_mul(ta, ps, cos_c)
                    tb = ph2.tile([128, 512], F32, tag="tb")
                    nc.vector.tensor_mul(tb, ps, sinsh_c)
                    tbs = ph2.tile([128, 512], F32, tag="tbs")
                    for g in range(4):
                        nc.sync.dma_start(
                            out=tbs[g * 32:(g + 1) * 32, :],
                            in_=tb[(g ^ 1) * 32:((g ^ 1) + 1) * 32, :])
                    if kind == "q":
                        gp_rope.tensor_add(rq[:, of, :], ta, tbs)
                    else:
                        gp_rope.tensor_add(ropek[:, of, cs], ta, tbs)
                else:
                    vtmp = ph2.tile([128, 512], F32R, tag="vtmp")
                    nc.scalar.copy(vtmp, ps)
                    vt = scr_ps.tile([128, 512], F32R, tag="scr")
                    for b in range(4):
                        nc.tensor.transpose(
                            vt[:, b * 128:(b + 1) * 128],
                            vtmp[:, b * 128:(b + 1) * 128], ident_sb)
                    nc.vector.tensor_copy(
                        vaug[:, c * 4:c * 4 + 4, of * 2:of * 2 + 2, 0:DH],
                        vt.rearrange("p (j h dd) -> p j h dd", j=4, h=2))

            if pending_wo is not None:
                _emit_wo(*pending_wo)
                pending_wo = None

            # ---------- attention for q-chunk c, both head pairs ----------
            # j-loop is software-pipelined: scores for j+1 issue before the
            # ctx matmuls of j, so the PE never waits on exp (scalar engine).
            nj = 4 * (c + 1)
            for p in range(2):
                ctxps = []
                for hi, cpool in ((0, ctxA_ps), (1, ctxB_ps)):
                    ctxps.append(cpool.tile([DH + 1, 512], F32,
                                            name=f"ctxp{hi}", tag=f"ctxp{hi}"))
                pend = None  # (j, ats) waiting for its ctx matmuls
                for j in range(nj):
                    dj = j - 4 * c
                    lo = max(dj, 0) * 128  # causally-valid q-column start
                    sps = []
                    for hi in range(2):
                        off = hi * DH
                        sp = s_ps.tile([128, 512], F32, name=f"sp{hi}",
                                       tag="sp")
                        nc.tensor.matmul(
                            sp[:, lo:512],
                            ropek[off:off + DH, p, j * 128:(j + 1) * 128],
                            rq[off:off + DH, p, lo:512],
                            start=True, stop=True, tile_position=(off, 0))
                        sps.append(sp)
                    ats = []
                    for hi in range(2):
                        a_t = ph3.tile([128, 512], F32R, name=f"a_t{hi}",
                                       tag=f"a_t{hi}")
                        nc.scalar.activation(
                            out=a_t[:, lo:512], in_=sps[hi][:, lo:512],
                            func=AF.Exp, scale=float(SCALE))
                        if dj >= 0:
                            gp_mask.tensor_mul(
                                a_t[:, lo:lo + 128],
                                a_t[:, lo:lo + 128], tri_sb)
                        ats.append(a_t)
                    if pend is not None:
                        pj, pats, plo = pend
                        for hi in range(2):
                            h = 2 * p + hi
                            nc.tensor.matmul(
                                ctxps[hi][:, plo:512],
                                vaug[:, pj, h, :], pats[hi][:, plo:512],
                                start=(pj == 0), stop=False)
                    pend = (j, ats, lo)
                pj, pats, plo = pend
                for hi in range(2):
                    h = 2 * p + hi
                    nc.tensor.matmul(
                        ctxps[hi][:, plo:512], vaug[:, pj, h, :],
                        pats[hi][:, plo:512],
                        start=(pj == 0), stop=True)
                for hi in range(2):
                    off = hi * DH
                    recip = ph3s.tile([1, 512], F32, tag="recip")
                    nc.vector.reciprocal(
                        out=recip, in_=ctxps[hi][DH:DH + 1, :])
                    dtmp = dsc.tile([1, 512], F32, tag="dtmp")
                    nc.sync.dma_start(out=dtmp, in_=recip)
                    rb = ph3s.tile([128, 512], F32, tag="rb")
                    bcast = bass.AP(tensor=dtmp.tensor, offset=dtmp.offset,
                                    ap=[[0, DH]] + list(dtmp.ap[1:]))
                    nc.sync.dma_start(out=rb[off:off + DH, :], in_=bcast)
                    dstc = cx[off:off + DH, p, :]
                    if hi == 0:
                        nc.vector.tensor_copy(dstc, ctxps[hi][0:DH, :])
                    else:
                        nc.scalar.copy(dstc, ctxps[hi][0:DH, :])
                    gp_norm.tensor_mul(dstc, dstc, rb[off:off + DH, :])
                if p == 0 and c + 1 < CH:
                    _emit_ln_tp(c + 1)
            pending_wo = (c, cx)
        _emit_wo(*pending_wo)


def make_in_maps(x, gamma, beta, Wq, Wkv, Wo):
    x = np.asarray(x, dtype=np.float32)
    gamma = np.asarray(gamma, dtype=np.float32)
    beta = np.asarray(beta, dtype=np.float32)
    Wq = np.asarray(Wq, dtype=np.float32)
    Wkv = np.asarray(Wkv, dtype=np.float32)
    Wo = np.asarray(Wo, dtype=np.float32)
    if np.any(beta != 0.0):
        raise NotImplementedError("nonzero beta not supported by this kernel")
    wq_f = gamma[:, None] * Wq                       # fold gamma into weights
    wk_f = gamma[:, None] * Wkv[:, :DIM]
    wv_f = gamma[:, None] * Wkv[:, DIM:]
    cos128, sinsh128 = _rope_tables()
    tri = np.triu(np.ones((128, 128), dtype=np.float32))  # valid: k <= q
    ident = np.eye(128, dtype=np.float32)
    in_maps = []
    for core in range(N_CORES):
        b, hg = divmod(core, 4)
        sl = slice(hg * HG, (hg + 1) * HG)
        in_maps.append({
            "x": np.ascontiguousarray(x[b]),
            "wq": np.ascontiguousarray(wq_f[:, sl]),
            "wk": np.ascontiguousarray(wk_f[:, sl]),
            "wv": np.ascontiguousarray(wv_f[:, sl]),
            "wo": np.ascontiguousarray(Wo[sl, :]),
            "cos": cos128,
            "sinsh": sinsh128,
            "tri": tri,
            "ident": ident,
            "onez": np.ones((128, 384), dtype=np.float32),
        })
    return in_maps


def kernel(x, gamma, beta, Wq, Wkv, Wo, _trace=False):
    in_maps = make_in_maps(x, gamma, beta, Wq, Wkv, Wo)
    if "nc" not in _CACHE:
        _CACHE["nc"] = build_nc()
    nc = _CACHE["nc"]
    res = bass_utils.run_bass_kernel_spmd(
        nc, in_maps, core_ids=list(range(N_CORES)), trace=_trace)
    out = np.zeros((2, N, DIM), dtype=np.float64)
    for core in range(N_CORES):
        b = core // 4
        out[b] += res.results[core]["out"].astype(np.float64)
    _CACHE["last_results"] = res
    return out.astype(np.float32)

